# revision 44
# baseline (speedup 1.0000x reference)
"""Inverse STFT (nn_InverseSTFT) as a Bass/Tile kernel on 8 TRN2 NeuronCores.

Math
----
Reference: one-sided stft -> full spectrum (conj symmetry), IDFT (K=1024),
overlap-add with hop=256, window-sum normalize, trim n_fft//2.

Two radix factorizations collapse the work:

1) hop = N/4: every folded-basis row is a single-frequency sinusoid, so
   w = 256j + r factors with coefficients in {-1,0,1} by f mod 4. The
   4-way overlap-add becomes a HOST-side shifted-add prefilter u (same
   bytes as x) + matmuls over K=1024 (U part) and K=512 (H part, where
   the H moving operand is just u shifted one segment).

2) f <-> 512-f pairing: basis rows satisfy g_{512-f}[r] = +/-(-1)^r
   g_f[r] (+ for cos-type, - for sin-type rows; same relation holds for
   the sigma-folded quadrature H rows). Host-side pairing w+/- = u_f +/-
   tau u_{512-f} splits the output into even/odd columns r with K halved
   per column. Net device work per 128-seg x 256-r tile: 12 matmuls of
   N=128 (vs 32 of N=256 naively) -- 5.3x less PE streaming than the
   direct form, at identical input bytes.

Device layout: 8 chunks of 128 rows each (singles used once, doubles
used twice: unshifted x g-basis and shifted x h-basis):
  E0,E1 (even-r singles)  ED0,ED1 (even-r doubles)
  O0,O1 (odd-r singles)   OD0,OD1 (odd-r doubles)
Each batch-0 chunk tile carries its own basis columns (one DMA delivers
everything that chunk's sweeps need). Even/odd chains write interleaved
PSUM columns (stride-2 matmul out APs) so the r-interleave happens in
the PE drain and eviction is a plain contiguous copy.

PSUM: two s-tiles (2 x 256 interleaved cols) per bank; the bank's first
matmul uses start=True (clears the bank), the other three chains run
all-start=False relying on has_written=0 -> overwrite (HW-validated).

Window-sum normalization = 0.25 folded into bases; per-partition fixup
on the two edge s-tiles. Output keeps segments s = 2..2002.

Sharding: pure data parallel, 2 batches per core.
"""

import numpy as np

import concourse.bass as bass
import concourse.mybir as mybir
from concourse.tile import TileContext
from concourse import bacc, bass_utils

N_FFT = 1024
HOP = 256
B = 16
T = 2000
NCORES = 8
NB = B // NCORES          # batches per core
KC = 8                    # signal chunks of 128 rows
SU = 2052                 # signal free size: i in [0, 2052), i <-> s = i-1
SEG = 2003                # total segments in un-trimmed output
OUT_SEGS = 2001           # segments s = 2..2002
NT = 16                   # s-tiles of 128 per batch (last has 81 valid rows)
OUT_LEN = OUT_SEGS * HOP  # 512256
SUS = SU + 128            # single-use chunk tile width (u + g basis)
SUD = SU + 256            # double-use chunk tile width (u + g + h basis)
DBL = (2, 3, 6, 7)        # chunk indices of doubles (ED0,ED1,OD0,OD1)

F32 = mybir.dt.float32
DT_IN = mybir.dt.bfloat16

import ml_dtypes

NP_IN = ml_dtypes.bfloat16


def _tables():
    """Row bookkeeping + per-chunk (128,128) g/h bases, 0.25 folded.

    Original folded-basis rows: k=0..512 cos-type (f=k), k=513..1023
    sin-type (f=k-512). Pair (f, 512-f) within type; tau=+1 cos, -1 sin.
    Chunks (row specs are (k, kp, tau); kp=None for self-pairs):
      0,1: singles-even  = [C-even-class w+ (129) | S-even w+ (127)]
      2,3: doubles       = [C-odd w (128) | S-odd w (128)]
      4,5: singles-odd   = [C-even w- (128) | S-even w- (127) | S-self]
      6,7: doubles       = same rows as 2,3 (w- variant)
    Even-r matmuls use chunks 0-3 (doubles: g at s, h at s-1); odd-r use
    chunks 4-7.
    """
    fk = np.concatenate([np.arange(513), np.arange(1, 512)])
    is_sin = np.concatenate([np.zeros(513, bool), np.ones(511, bool)])
    kkk = np.arange(1024)
    gamma = np.where((kkk == 0) | (kkk == 512), 1.0 / 1024, 2.0 / 1024)
    gamma = np.where(is_sin, -2.0 / 1024, gamma)
    r = np.arange(256)
    th = 2 * np.pi * np.outer(fk, r) / 1024.0
    g = np.where(is_sin[:, None], gamma[:, None] * np.sin(th),
                 gamma[:, None] * np.cos(th))
    h = np.where(is_sin[:, None], gamma[:, None] * np.cos(th),
                 -gamma[:, None] * np.sin(th))
    cls = fk % 4
    sigma = np.where(cls == 1, 1.0, np.where(cls == 3, -1.0, 0.0))
    Ub = g * 0.25
    Hb = h * sigma[:, None] * 0.25

    def row_of(typ, f):
        return f if typ == "C" else 512 + f

    pairs = {"CE": [], "SE": [], "CO": [], "SO": []}
    for typ in ("C", "S"):
        for f in range(1 if typ == "S" else 0, 256):
            k = row_of(typ, f)
            key = typ + ("O" if cls[k] % 2 else "E")
            pairs[key].append((k, row_of(typ, 512 - f),
                               1.0 if typ == "C" else -1.0))
    cself = (row_of("C", 256), None, 0.0)
    sself = (row_of("S", 256), None, 0.0)

    singles_e = pairs["CE"] + [cself] + pairs["SE"]          # 256
    doubles = pairs["CO"] + pairs["SO"]                      # 256
    singles_o = pairs["CE"] + pairs["SE"] + [sself]          # 256
    chunk_rows = [singles_e[:128], singles_e[128:],
                  doubles[:128], doubles[128:],
                  singles_o[:128], singles_o[128:],
                  doubles[:128], doubles[128:]]
    chunk_sign = [+1, +1, +1, +1, -1, -1, -1, -1]   # w+ or w- variant
    chunk_par = [0, 0, 0, 0, 1, 1, 1, 1]            # r parity served

    gbas, hbas = [], []
    for ci in range(KC):
        par = chunk_par[ci]
        gb = np.zeros((128, 128))
        hb = np.zeros((128, 128)) if ci in DBL else None
        for i, (k, kp, tau) in enumerate(chunk_rows[ci]):
            gb[i] = Ub[k][par::2]
            if hb is not None:
                hb[i] = Hb[k][par::2]
        gbas.append(gb.astype(NP_IN))
        hbas.append(None if hb is None else hb.astype(NP_IN))

    ka = np.zeros((KC, 128), np.int64)
    kb = np.zeros((KC, 128), np.int64)
    co = np.zeros((KC, 128), np.float32)
    for ci in range(KC):
        for i, (k, kp, tau) in enumerate(chunk_rows[ci]):
            ka[ci, i] = k
            kb[ci, i] = k if kp is None else kp
            co[ci, i] = 0.0 if kp is None else chunk_sign[ci] * tau
    cls_tab = cls
    return ka, kb, co, cls_tab, gbas, hbas


_KA, _KB, _CO, _CLS, _GBAS, _HBAS = _tables()


def _make_scales() -> np.ndarray:
    """(128, 2) per-partition wss fixup (on top of the 0.25 in the bases).

    col 0 -> first s-tile (s = 2..129): s=2 has 3 frames -> 4/3.
    col 1 -> last s-tile (s = 1922..2002): s=2000 -> 4/3, 2001 -> 2, 2002 -> 4.
    """
    sc = np.ones((128, 2), np.float32)
    sc[0, 0] = np.float32(4.0) / np.float32(3.0)
    sc[78, 1] = np.float32(4.0) / np.float32(3.0)
    sc[79, 1] = 2.0
    sc[80, 1] = 4.0
    return sc


def _prep_w(stft: np.ndarray) -> np.ndarray:
    """(16,513,2000,2) f32 -> (16, KC, 128, SU) paired signals, bf16.

    u[k, i] <-> s = i-1 (x zero outside [0, T)); chunk rows are
    w = u[ka] + co * u[kb], computed in f32, cast to bf16.
    """
    re = stft[:, :, :, 0]
    im = stft[:, 1:512, :, 1]
    xk = np.concatenate([re, im], axis=1)          # (B, 1024, T)
    xp = np.zeros((B, 1024, 2056), np.float32)
    xp[:, :, 4 : 4 + T] = xk
    x0 = xp[:, :, 3 : 3 + SU]
    x1 = xp[:, :, 2 : 2 + SU]
    x2 = xp[:, :, 1 : 1 + SU]
    x3 = xp[:, :, 0 : SU]
    u = np.empty((B, 1024, SU), np.float32)
    m0 = _CLS == 0
    m2 = _CLS == 2
    modd = (_CLS % 2) == 1
    u[:, m0] = (x0 + x1 + x2 + x3)[:, m0]
    u[:, m2] = (x0 - x1 + x2 - x3)[:, m2]
    u[:, modd] = (x0 - x2)[:, modd]
    W = (u[:, _KA.reshape(-1)] +
         _CO.reshape(1, -1, 1) * u[:, _KB.reshape(-1)])
    return W.reshape(B, KC, 128, SU).astype(NP_IN)


def _fuse_inputs(W: np.ndarray):
    """Per-core inputs: batch-0 chunk tiles carry their basis columns;
    batch-1 ships as one wide partition-major block."""
    ncores_in = W.shape[0] // NB
    w0s = np.zeros((ncores_in, 4, 128, SUS), NP_IN)   # singles 0,1,4,5
    w0d = np.zeros((ncores_in, 4, 128, SUD), NP_IN)   # doubles 2,3,6,7
    singles = [c for c in range(KC) if c not in DBL]
    for i, ci in enumerate(singles):
        w0s[:, i, :, :SU] = W[0::NB, ci]
        w0s[:, i, :, SU:] = _GBAS[ci]
    for i, ci in enumerate(DBL):
        w0d[:, i, :, :SU] = W[0::NB, ci]
        w0d[:, i, :, SU : SU + 128] = _GBAS[ci]
        w0d[:, i, :, SU + 128 :] = _HBAS[ci]
    w1 = W[1::NB].reshape(ncores_in, 2, 4, 128, SU).transpose(
        0, 1, 3, 2, 4).reshape(ncores_in, 2, 128, 4 * SU)
    return (np.ascontiguousarray(w0s), np.ascontiguousarray(w0d),
            np.ascontiguousarray(w1))


# sweeps: (chunk, basis, shift). Chunks 0-3 accumulate the even-r chain,
# 4-7 the odd-r chain; doubles contribute g at s and h at s-1.
SWEEPS = [(0, "g", 0), (1, "g", 0), (2, "g", 0), (2, "h", 1),
          (3, "g", 0), (3, "h", 1),
          (4, "g", 0), (5, "g", 0), (6, "g", 0), (6, "h", 1),
          (7, "g", 0), (7, "h", 1)]
EVEN_LAST = 5   # sweep index of the even chain's final contribution


def _build_nc() -> bass.Bass:
    nc = bacc.Bacc()
    w0s_in = nc.dram_tensor("w0s_in", [4, 128, SUS], DT_IN, kind="ExternalInput")
    w0d_in = nc.dram_tensor("w0d_in", [4, 128, SUD], DT_IN, kind="ExternalInput")
    w1_in = nc.dram_tensor("w1_in", [2, 128, 4 * SU], DT_IN, kind="ExternalInput")
    scale_in = nc.dram_tensor("scale_in", [128, 2], F32, kind="ExternalInput")
    # output ships as bf16 (halves write traffic; ~0.17% quantization
    # against a 2e-2 gate) and is upcast to f32 on the host
    out = nc.dram_tensor("out", [NB, OUT_SEGS, HOP], DT_IN, kind="ExternalOutput")

    with TileContext(nc) as tc:
        with (
            tc.tile_pool(name="up", bufs=1) as u_pool,
            tc.tile_pool(name="sp", bufs=1) as s_pool,
            tc.tile_pool(name="wu", bufs=1) as wu_pool,
            tc.tile_pool(name="ev", bufs=2) as ev_pool,
            tc.tile_pool(name="ps", bufs=8, space="PSUM") as psum_pool,
        ):
            # one DMA per batch-0 chunk (8 = all HWDGE sem lanes), in
            # sweep-consumption order, alternating the SP/ACT rings so
            # triggers (~650ns engine issue each) pipeline 2x as fast.
            w_sb = [[None] * KC for _ in range(NB)]
            singles = [c for c in range(KC) if c not in DBL]
            for ci in range(KC):
                if ci in DBL:
                    tile = u_pool.tile([128, SUD], DT_IN, name=f"w0_{ci}",
                                       tag=f"w0_{ci}")
                    src = w0d_in[DBL.index(ci)]
                else:
                    tile = u_pool.tile([128, SUS], DT_IN, name=f"w0_{ci}",
                                       tag=f"w0_{ci}")
                    src = w0s_in[singles.index(ci)]
                eng = nc.sync if ci % 2 == 0 else nc.scalar
                eng.dma_start(tile[:, :], src)
                w_sb[0][ci] = tile

            # batch-1 as TWO 4-chunk blocks: only 2 queues compete with
            # batch-0's critical-path transfers (vs 8), and each block
            # lands just before batch-1's sweeps consume it.
            for blk in range(2):
                wt = u_pool.tile([128, 4 * SU], DT_IN, name=f"w1b{blk}",
                                 tag=f"w1b{blk}")
                eng = nc.sync if blk == 0 else nc.scalar
                eng.dma_start(wt[:, :], w1_in[blk])
                for i in range(4):
                    w_sb[1][4 * blk + i] = wt[:, i * SU : (i + 1) * SU]

            def gb_ap(ci):
                return w_sb[0][ci][:, SU : SU + 128]

            def hb_ap(ci):
                return w_sb[0][ci][:, SU + 128 : SU + 256]

            scale_sb = s_pool.tile([128, 2], F32, name="scale_sb", tag="scale_sb")
            scale_wu = s_pool.tile([128, 2], F32, name="scale_wu", tag="scale_wu")
            nc.gpsimd.dma_start(scale_sb[:, :], scale_in[:, :])
            nc.scalar.copy(scale_wu[:, :], scale_sb[:, :])

            # PE warm-up: ~3.5us of dummy matmuls on zeroed scratch while
            # the first chunk is in flight -> HAM reaches 8/8 (2.4 GHz)
            # before the first real matmul.
            wu_w = wu_pool.tile([128, 128], DT_IN, name="wu_w", tag="wu_w")
            wu_r = wu_pool.tile([128, 256], DT_IN, name="wu_r", tag="wu_r")
            nc.vector.memset(wu_w[:, :], 0)
            nc.vector.memset(wu_r[:, :], 0)
            wu_ps = psum_pool.tile([128, 2 * HOP], F32, name="wu_ps", tag="psum")
            for i in range(16):
                nc.tensor.matmul(
                    wu_ps[:, :HOP], wu_w[:, :], wu_r[:, :],
                    start=(i == 0), stop=(i == 15),
                )

            # phases: batch 0 as one 16-s-tile phase (chunk-major sweeps
            # track DMA arrival); batch 1 as two 8-s-tile phases so the
            # second phase's matmuls hide the first phase's output writes.
            # Two s-tiles per PSUM bank (256 interleaved cols each); only
            # the bank's very first matmul uses start=True, the other 3
            # chains rely on has_written=0 -> overwrite.
            phases = [(0, list(range(0, 16))),
                      (1, list(range(0, 16)))]
            for pidx, (b, sts) in enumerate(phases):
                banks = {
                    k: psum_pool.tile([128, 2 * HOP], F32,
                                      name=f"ps{pidx}_{k}", tag="psum")
                    for k in sorted({st // 2 for st in sts})
                }

                def ps_ap(st, par):
                    base = 256 * (st % 2) + 128 * par
                    return banks[st // 2][:, base : base + 128]

                def emit_mm(ci, bas, shift, st, start, stop):
                    par = 0 if ci < 4 else 1
                    rhs = gb_ap(ci) if bas == "g" else hb_ap(ci)
                    s0 = 2 + 128 * st
                    lo = s0 + 1 - shift
                    nc.tensor.matmul(
                        ps_ap(st, par),
                        w_sb[b][ci][:, lo : lo + 128],
                        rhs,
                        start=start, stop=stop,
                        skip_group_check=True,
                    )

                def emit_evict(st):
                    # both parity regions are adjacent in PSUM, so ONE
                    # copy per s-tile interleaves them into the staging
                    # tile via a rearranged destination AP. Edge s-tiles
                    # get the wss fixup scale on ScalarE; plain copies
                    # alternate VectorE/ScalarE.
                    base = 256 * (st % 2)
                    srcp = banks[st // 2][:, base : base + 256].rearrange(
                        "p (par i) -> p par i", par=2)
                    dst = ev[:, st * HOP : (st + 1) * HOP].rearrange(
                        "p (i par) -> p par i", par=2)
                    if st == 0:
                        nc.scalar.mul(dst, srcp, scale_sb[:, 0:1])
                    elif st == NT - 1:
                        nc.scalar.mul(dst, srcp, scale_sb[:, 1:2])
                    elif st % 2 == 0:
                        nc.vector.tensor_copy(dst, srcp)
                    else:
                        nc.scalar.copy(dst, srcp)

                def emit_out_piece(st_lo, st_hi, eng):
                    # contiguous s-tiles [st_lo, st_hi] as one DMA
                    k = st_hi - st_lo + 1
                    seg0 = 128 * st_lo
                    if st_hi == NT - 1:
                        eng.dma_start(
                            out[b, 1920:2001, :], ev[:81, 15 * HOP : 16 * HOP]
                        )
                    elif k == 1:
                        eng.dma_start(
                            out[b, seg0 : seg0 + 128, :],
                            ev[:, st_lo * HOP : (st_lo + 1) * HOP],
                        )
                    else:
                        eng.dma_start(
                            out[b, seg0 : seg0 + 128 * k, :].rearrange(
                                "(k p) r -> p k r", p=128),
                            ev[:, st_lo * HOP : (st_hi + 1) * HOP].rearrange(
                                "p (k r) -> p k r", k=k),
                        )

                ev = ev_pool.tile([128, NT * HOP], DT_IN, name="ev", tag="ev")
                # chunk-major sweeps 0..9 track DMA arrival; the final two
                # sweeps run s-tile-major with the eviction fused per
                # s-tile so the evict + output pipeline drains immediately
                # behind the last matmuls.
                for si, (ci, bas, shift) in enumerate(SWEEPS[:-4]):
                    for st in sts:
                        emit_mm(ci, bas, shift, st,
                                start=(si == 0 and st % 2 == 0),
                                stop=(si == EVEN_LAST))
                for k in sorted(banks):
                    # the last four sweeps run bank-major with the
                    # eviction fused per bank: each bank's chains finish
                    # 8 matmuls apart, so the eviction stream keeps pace
                    # with the matmul stream and the tail drains flat
                    for st in (2 * k, 2 * k + 1):
                        for (ci, bas, shift) in SWEEPS[-4:]:
                            emit_mm(ci, bas, shift, st, start=False,
                                    stop=(ci == 7 and bas == "h"))
                    emit_evict(2 * k)
                    emit_evict(2 * k + 1)
                pieces = {
                    (0, 16): [(0, 3, nc.sync), (4, 7, nc.scalar),
                              (8, 11, nc.sync), (12, 13, nc.scalar),
                              (14, 14, nc.sync), (15, 15, nc.scalar)],
                    (0, 8): [(0, 3, nc.sync), (4, 7, nc.scalar)],
                    (8, 16): [(8, 11, nc.sync), (12, 13, nc.scalar),
                              (14, 14, nc.sync), (15, 15, nc.scalar)],
                }[(sts[0], sts[-1] + 1)]
                for (lo, hi, eng) in pieces:
                    emit_out_piece(lo, hi, eng)
    nc.finalize()
    return nc


def _run(inputs: dict, trace: bool = False):
    stft = np.asarray(inputs["stft_matrix"], dtype=np.float32)
    W = _prep_w(stft)
    w0s, w0d, w1 = _fuse_inputs(W)
    scales = _make_scales()
    in_maps = [
        {"w0s_in": w0s[c], "w0d_in": w0d[c], "w1_in": w1[c],
         "scale_in": scales}
        for c in range(NCORES)
    ]
    nc = _build_nc()
    res = bass_utils.run_bass_kernel_spmd(
        nc, in_maps, core_ids=list(range(NCORES)), trace=trace
    )
    out = np.concatenate(
        [res.results[c]["out"].astype(np.float32).reshape(NB, OUT_LEN)
         for c in range(NCORES)], axis=0
    )
    return out, res


def kernel(**inputs) -> np.ndarray:
    out, _ = _run(inputs, trace=False)
    return out


# revision 45
# speedup vs baseline: 1.2387x; 1.2387x over previous
"""Inverse STFT (nn_InverseSTFT) as a Bass/Tile kernel on 8 TRN2 NeuronCores.

Math
----
Reference: one-sided stft -> full spectrum (conj symmetry), IDFT (K=1024),
overlap-add with hop=256, window-sum normalize, trim n_fft//2.

Two radix factorizations collapse the work:

1) hop = N/4: every folded-basis row is a single-frequency sinusoid, so
   w = 256j + r factors with coefficients in {-1,0,1} by f mod 4. The
   4-way overlap-add becomes a HOST-side shifted-add prefilter u (same
   bytes as x) + matmuls over K=1024 (U part) and K=512 (H part, where
   the H moving operand is just u shifted one segment).

2) f <-> 512-f pairing: basis rows satisfy g_{512-f}[r] = +/-(-1)^r
   g_f[r] (+ for cos-type, - for sin-type rows; same relation holds for
   the sigma-folded quadrature H rows). Host-side pairing w+/- = u_f +/-
   tau u_{512-f} splits the output into even/odd columns r with K halved
   per column. Net device work per 128-seg x 256-r tile: 12 matmuls of
   N=128 (vs 32 of N=256 naively) -- 5.3x less PE streaming than the
   direct form, at identical input bytes.

Device layout: 8 chunks of 128 rows each (singles used once, doubles
used twice: unshifted x g-basis and shifted x h-basis):
  E0,E1 (even-r singles)  ED0,ED1 (even-r doubles)
  O0,O1 (odd-r singles)   OD0,OD1 (odd-r doubles)
Each batch-0 chunk tile carries its own basis columns (one DMA delivers
everything that chunk's sweeps need). Even/odd chains write interleaved
PSUM columns (stride-2 matmul out APs) so the r-interleave happens in
the PE drain and eviction is a plain contiguous copy.

PSUM: two s-tiles (2 x 256 interleaved cols) per bank; the bank's first
matmul uses start=True (clears the bank), the other three chains run
all-start=False relying on has_written=0 -> overwrite (HW-validated).

Window-sum normalization = 0.25 folded into bases; per-partition fixup
on the two edge s-tiles. Output keeps segments s = 2..2002.

Sharding: pure data parallel, 2 batches per core.
"""

import numpy as np

import concourse.bass as bass
import concourse.mybir as mybir
from concourse.tile import TileContext
from concourse import bacc, bass_utils

N_FFT = 1024
HOP = 256
B = 16
T = 2000
NCORES = 8
NB = B // NCORES          # batches per core
KC = 8                    # signal chunks of 128 rows
SU = 2052                 # signal free size: i in [0, 2052), i <-> s = i-1
SEG = 2003                # total segments in un-trimmed output
OUT_SEGS = 2001           # segments s = 2..2002
NT = 16                   # s-tiles of 128 per batch (last has 81 valid rows)
OUT_LEN = OUT_SEGS * HOP  # 512256
SUS = SU + 128            # single-use chunk tile width (u + g basis)
SUD = SU + 256            # double-use chunk tile width (u + g + h basis)
DBL = (2, 3, 6, 7)        # chunk indices of doubles (ED0,ED1,OD0,OD1)

F32 = mybir.dt.float32
DT_IN = mybir.dt.bfloat16

import ml_dtypes

NP_IN = ml_dtypes.bfloat16


def _tables():
    """Row bookkeeping + per-chunk (128,128) g/h bases, 0.25 folded.

    Original folded-basis rows: k=0..512 cos-type (f=k), k=513..1023
    sin-type (f=k-512). Pair (f, 512-f) within type; tau=+1 cos, -1 sin.
    Chunks (row specs are (k, kp, tau); kp=None for self-pairs):
      0,1: singles-even  = [C-even-class w+ (129) | S-even w+ (127)]
      2,3: doubles       = [C-odd w (128) | S-odd w (128)]
      4,5: singles-odd   = [C-even w- (128) | S-even w- (127) | S-self]
      6,7: doubles       = same rows as 2,3 (w- variant)
    Even-r matmuls use chunks 0-3 (doubles: g at s, h at s-1); odd-r use
    chunks 4-7.
    """
    fk = np.concatenate([np.arange(513), np.arange(1, 512)])
    is_sin = np.concatenate([np.zeros(513, bool), np.ones(511, bool)])
    kkk = np.arange(1024)
    gamma = np.where((kkk == 0) | (kkk == 512), 1.0 / 1024, 2.0 / 1024)
    gamma = np.where(is_sin, -2.0 / 1024, gamma)
    r = np.arange(256)
    th = 2 * np.pi * np.outer(fk, r) / 1024.0
    g = np.where(is_sin[:, None], gamma[:, None] * np.sin(th),
                 gamma[:, None] * np.cos(th))
    h = np.where(is_sin[:, None], gamma[:, None] * np.cos(th),
                 -gamma[:, None] * np.sin(th))
    cls = fk % 4
    sigma = np.where(cls == 1, 1.0, np.where(cls == 3, -1.0, 0.0))
    Ub = g * 0.25
    Hb = h * sigma[:, None] * 0.25

    def row_of(typ, f):
        return f if typ == "C" else 512 + f

    pairs = {"CE": [], "SE": [], "CO": [], "SO": []}
    for typ in ("C", "S"):
        for f in range(1 if typ == "S" else 0, 256):
            k = row_of(typ, f)
            key = typ + ("O" if cls[k] % 2 else "E")
            pairs[key].append((k, row_of(typ, 512 - f),
                               1.0 if typ == "C" else -1.0))
    cself = (row_of("C", 256), None, 0.0)
    sself = (row_of("S", 256), None, 0.0)

    singles_e = pairs["CE"] + [cself] + pairs["SE"]          # 256
    doubles = pairs["CO"] + pairs["SO"]                      # 256
    singles_o = pairs["CE"] + pairs["SE"] + [sself]          # 256
    chunk_rows = [singles_e[:128], singles_e[128:],
                  doubles[:128], doubles[128:],
                  singles_o[:128], singles_o[128:],
                  doubles[:128], doubles[128:]]
    chunk_sign = [+1, +1, +1, +1, -1, -1, -1, -1]   # w+ or w- variant
    chunk_par = [0, 0, 0, 0, 1, 1, 1, 1]            # r parity served

    gbas, hbas = [], []
    for ci in range(KC):
        par = chunk_par[ci]
        gb = np.zeros((128, 128))
        hb = np.zeros((128, 128)) if ci in DBL else None
        for i, (k, kp, tau) in enumerate(chunk_rows[ci]):
            gb[i] = Ub[k][par::2]
            if hb is not None:
                hb[i] = Hb[k][par::2]
        gbas.append(gb.astype(NP_IN))
        hbas.append(None if hb is None else hb.astype(NP_IN))

    ka = np.zeros((KC, 128), np.int64)
    kb = np.zeros((KC, 128), np.int64)
    co = np.zeros((KC, 128), np.float32)
    for ci in range(KC):
        for i, (k, kp, tau) in enumerate(chunk_rows[ci]):
            ka[ci, i] = k
            kb[ci, i] = k if kp is None else kp
            co[ci, i] = 0.0 if kp is None else chunk_sign[ci] * tau
    cls_tab = cls
    return ka, kb, co, cls_tab, gbas, hbas


_KA, _KB, _CO, _CLS, _GBAS, _HBAS = _tables()


def _make_scales() -> np.ndarray:
    """(128, 2) per-partition wss fixup (on top of the 0.25 in the bases).

    col 0 -> first s-tile (s = 2..129): s=2 has 3 frames -> 4/3.
    col 1 -> last s-tile (s = 1922..2002): s=2000 -> 4/3, 2001 -> 2, 2002 -> 4.
    """
    sc = np.ones((128, 2), np.float32)
    sc[0, 0] = np.float32(4.0) / np.float32(3.0)
    sc[78, 1] = np.float32(4.0) / np.float32(3.0)
    sc[79, 1] = 2.0
    sc[80, 1] = 4.0
    return sc


def _prep_w(stft: np.ndarray) -> np.ndarray:
    """(16,513,2000,2) f32 -> (16, KC, 128, SU) paired signals, bf16.

    u[k, i] <-> s = i-1 (x zero outside [0, T)); chunk rows are
    w = u[ka] + co * u[kb], computed in f32, cast to bf16.
    """
    re = stft[:, :, :, 0]
    im = stft[:, 1:512, :, 1]
    xk = np.concatenate([re, im], axis=1)          # (B, 1024, T)
    xp = np.zeros((B, 1024, 2056), np.float32)
    xp[:, :, 4 : 4 + T] = xk
    x0 = xp[:, :, 3 : 3 + SU]
    x1 = xp[:, :, 2 : 2 + SU]
    x2 = xp[:, :, 1 : 1 + SU]
    x3 = xp[:, :, 0 : SU]
    u = np.empty((B, 1024, SU), np.float32)
    m0 = _CLS == 0
    m2 = _CLS == 2
    modd = (_CLS % 2) == 1
    u[:, m0] = (x0 + x1 + x2 + x3)[:, m0]
    u[:, m2] = (x0 - x1 + x2 - x3)[:, m2]
    u[:, modd] = (x0 - x2)[:, modd]
    W = (u[:, _KA.reshape(-1)] +
         _CO.reshape(1, -1, 1) * u[:, _KB.reshape(-1)])
    return W.reshape(B, KC, 128, SU).astype(NP_IN)


def _fuse_inputs(W: np.ndarray):
    """Per-core inputs: batch-0 chunk tiles carry their basis columns;
    batch-1 ships as one wide partition-major block."""
    ncores_in = W.shape[0] // NB
    w0s = np.zeros((ncores_in, 4, 128, SUS), NP_IN)   # singles 0,1,4,5
    w0d = np.zeros((ncores_in, 4, 128, SUD), NP_IN)   # doubles 2,3,6,7
    singles = [c for c in range(KC) if c not in DBL]
    for i, ci in enumerate(singles):
        w0s[:, i, :, :SU] = W[0::NB, ci]
        w0s[:, i, :, SU:] = _GBAS[ci]
    for i, ci in enumerate(DBL):
        w0d[:, i, :, :SU] = W[0::NB, ci]
        w0d[:, i, :, SU : SU + 128] = _GBAS[ci]
        w0d[:, i, :, SU + 128 :] = _HBAS[ci]
    w1 = W[1::NB]
    return (np.ascontiguousarray(w0s), np.ascontiguousarray(w0d),
            np.ascontiguousarray(w1))


# sweeps: (chunk, basis, shift). Chunks 0-3 accumulate the even-r chain,
# 4-7 the odd-r chain; doubles contribute g at s and h at s-1.
SWEEPS = [(0, "g", 0), (1, "g", 0), (2, "g", 0), (2, "h", 1),
          (3, "g", 0), (3, "h", 1),
          (4, "g", 0), (5, "g", 0), (6, "g", 0), (6, "h", 1),
          (7, "g", 0), (7, "h", 1)]
EVEN_LAST = 5   # sweep index of the even chain's final contribution


def _build_nc() -> bass.Bass:
    nc = bacc.Bacc()
    w0s_in = nc.dram_tensor("w0s_in", [4, 128, SUS], DT_IN, kind="ExternalInput")
    w0d_in = nc.dram_tensor("w0d_in", [4, 128, SUD], DT_IN, kind="ExternalInput")
    w1_in = nc.dram_tensor("w1_in", [KC, 128, SU], DT_IN, kind="ExternalInput")
    scale_in = nc.dram_tensor("scale_in", [128, 2], F32, kind="ExternalInput")
    # output ships as bf16 (halves write traffic; ~0.17% quantization
    # against a 2e-2 gate) and is upcast to f32 on the host
    out = nc.dram_tensor("out", [NB, OUT_SEGS, HOP], DT_IN, kind="ExternalOutput")

    with TileContext(nc) as tc:
        with (
            tc.tile_pool(name="up", bufs=1) as u_pool,
            tc.tile_pool(name="sp", bufs=1) as s_pool,
            tc.tile_pool(name="wu", bufs=1) as wu_pool,
            tc.tile_pool(name="ev", bufs=2) as ev_pool,
            tc.tile_pool(name="ps", bufs=8, space="PSUM") as psum_pool,
        ):
            # one DMA per batch-0 chunk (8 = all HWDGE sem lanes), in
            # sweep-consumption order, alternating the SP/ACT rings so
            # triggers (~650ns engine issue each) pipeline 2x as fast.
            w_sb = [[None] * KC for _ in range(NB)]
            singles = [c for c in range(KC) if c not in DBL]
            for ci in range(KC):
                if ci in DBL:
                    tile = u_pool.tile([128, SUD], DT_IN, name=f"w0_{ci}",
                                       tag=f"w0_{ci}")
                    src = w0d_in[DBL.index(ci)]
                else:
                    tile = u_pool.tile([128, SUS], DT_IN, name=f"w0_{ci}",
                                       tag=f"w0_{ci}")
                    src = w0s_in[singles.index(ci)]
                eng = nc.sync if ci % 2 == 0 else nc.scalar
                eng.dma_start(tile[:, :], src)
                w_sb[0][ci] = tile

            # batch-1 chunks as 8 DMAs in consumption order: triggers
            # queue behind batch-0's on the HWDGE sem lanes, and batch-1's
            # chunk-major sweeps consume them as they arrive.
            for ci in range(KC):
                ut = u_pool.tile([128, SU], DT_IN, name=f"w1_{ci}",
                                 tag=f"w1_{ci}")
                eng = nc.sync if ci % 2 == 0 else nc.scalar
                eng.dma_start(ut[:, :], w1_in[ci])
                w_sb[1][ci] = ut

            def gb_ap(ci):
                return w_sb[0][ci][:, SU : SU + 128]

            def hb_ap(ci):
                return w_sb[0][ci][:, SU + 128 : SU + 256]

            scale_sb = s_pool.tile([128, 2], F32, name="scale_sb", tag="scale_sb")
            scale_wu = s_pool.tile([128, 2], F32, name="scale_wu", tag="scale_wu")
            nc.gpsimd.dma_start(scale_sb[:, :], scale_in[:, :])
            nc.scalar.copy(scale_wu[:, :], scale_sb[:, :])

            # PE warm-up: ~3.5us of dummy matmuls on zeroed scratch while
            # the first chunk is in flight -> HAM reaches 8/8 (2.4 GHz)
            # before the first real matmul.
            wu_w = wu_pool.tile([128, 128], DT_IN, name="wu_w", tag="wu_w")
            wu_r = wu_pool.tile([128, 256], DT_IN, name="wu_r", tag="wu_r")
            nc.vector.memset(wu_w[:, :], 0)
            nc.vector.memset(wu_r[:, :], 0)
            wu_ps = psum_pool.tile([128, 2 * HOP], F32, name="wu_ps", tag="psum")
            for i in range(16):
                nc.tensor.matmul(
                    wu_ps[:, :HOP], wu_w[:, :], wu_r[:, :],
                    start=(i == 0), stop=(i == 15),
                )

            # phases: batch 0 as one 16-s-tile phase (chunk-major sweeps
            # track DMA arrival); batch 1 as two 8-s-tile phases so the
            # second phase's matmuls hide the first phase's output writes.
            # Two s-tiles per PSUM bank (256 interleaved cols each); only
            # the bank's very first matmul uses start=True, the other 3
            # chains rely on has_written=0 -> overwrite.
            phases = [(0, list(range(0, 16))),
                      (1, list(range(0, 16)))]
            for pidx, (b, sts) in enumerate(phases):
                banks = {
                    k: psum_pool.tile([128, 2 * HOP], F32,
                                      name=f"ps{pidx}_{k}", tag="psum")
                    for k in sorted({st // 2 for st in sts})
                }

                def ps_ap(st, par):
                    base = 256 * (st % 2) + 128 * par
                    return banks[st // 2][:, base : base + 128]

                def emit_mm(ci, bas, shift, st, start, stop):
                    par = 0 if ci < 4 else 1
                    rhs = gb_ap(ci) if bas == "g" else hb_ap(ci)
                    s0 = 2 + 128 * st
                    lo = s0 + 1 - shift
                    nc.tensor.matmul(
                        ps_ap(st, par),
                        w_sb[b][ci][:, lo : lo + 128],
                        rhs,
                        start=start, stop=stop,
                        skip_group_check=True,
                    )

                def emit_evict(st):
                    # both parity regions are adjacent in PSUM, so ONE
                    # copy per s-tile interleaves them into the staging
                    # tile via a rearranged destination AP. Edge s-tiles
                    # get the wss fixup scale on ScalarE; plain copies
                    # alternate VectorE/ScalarE.
                    base = 256 * (st % 2)
                    srcp = banks[st // 2][:, base : base + 256].rearrange(
                        "p (par i) -> p par i", par=2)
                    dst = ev[:, st * HOP : (st + 1) * HOP].rearrange(
                        "p (i par) -> p par i", par=2)
                    if st == 0:
                        nc.scalar.mul(dst, srcp, scale_sb[:, 0:1])
                    elif st == NT - 1:
                        nc.scalar.mul(dst, srcp, scale_sb[:, 1:2])
                    elif st % 2 == 0:
                        nc.vector.tensor_copy(dst, srcp)
                    else:
                        nc.scalar.copy(dst, srcp)

                def emit_out_piece(st_lo, st_hi, eng):
                    # contiguous s-tiles [st_lo, st_hi] as one DMA
                    k = st_hi - st_lo + 1
                    seg0 = 128 * st_lo
                    if st_hi == NT - 1:
                        eng.dma_start(
                            out[b, 1920:2001, :], ev[:81, 15 * HOP : 16 * HOP]
                        )
                    elif k == 1:
                        eng.dma_start(
                            out[b, seg0 : seg0 + 128, :],
                            ev[:, st_lo * HOP : (st_lo + 1) * HOP],
                        )
                    else:
                        eng.dma_start(
                            out[b, seg0 : seg0 + 128 * k, :].rearrange(
                                "(k p) r -> p k r", p=128),
                            ev[:, st_lo * HOP : (st_hi + 1) * HOP].rearrange(
                                "p (k r) -> p k r", k=k),
                        )

                ev = ev_pool.tile([128, NT * HOP], DT_IN, name="ev", tag="ev")
                # chunk-major sweeps 0..9 track DMA arrival; the final two
                # sweeps run s-tile-major with the eviction fused per
                # s-tile so the evict + output pipeline drains immediately
                # behind the last matmuls.
                for si, (ci, bas, shift) in enumerate(SWEEPS[:-4]):
                    for st in sts:
                        emit_mm(ci, bas, shift, st,
                                start=(si == 0 and st % 2 == 0),
                                stop=(si == EVEN_LAST))
                for k in sorted(banks):
                    # the last four sweeps run bank-major with the
                    # eviction fused per bank: each bank's chains finish
                    # 8 matmuls apart, so the eviction stream keeps pace
                    # with the matmul stream and the tail drains flat
                    for st in (2 * k, 2 * k + 1):
                        for (ci, bas, shift) in SWEEPS[-4:]:
                            emit_mm(ci, bas, shift, st, start=False,
                                    stop=(ci == 7 and bas == "h"))
                    emit_evict(2 * k)
                    emit_evict(2 * k + 1)
                pieces = {
                    (0, 16): [(0, 3, nc.sync), (4, 7, nc.scalar),
                              (8, 11, nc.sync), (12, 13, nc.scalar),
                              (14, 14, nc.sync), (15, 15, nc.scalar)],
                    (0, 8): [(0, 3, nc.sync), (4, 7, nc.scalar)],
                    (8, 16): [(8, 11, nc.sync), (12, 13, nc.scalar),
                              (14, 14, nc.sync), (15, 15, nc.scalar)],
                }[(sts[0], sts[-1] + 1)]
                for (lo, hi, eng) in pieces:
                    emit_out_piece(lo, hi, eng)
    nc.finalize()
    return nc


def _run(inputs: dict, trace: bool = False):
    stft = np.asarray(inputs["stft_matrix"], dtype=np.float32)
    W = _prep_w(stft)
    w0s, w0d, w1 = _fuse_inputs(W)
    scales = _make_scales()
    in_maps = [
        {"w0s_in": w0s[c], "w0d_in": w0d[c], "w1_in": w1[c],
         "scale_in": scales}
        for c in range(NCORES)
    ]
    nc = _build_nc()
    res = bass_utils.run_bass_kernel_spmd(
        nc, in_maps, core_ids=list(range(NCORES)), trace=trace
    )
    out = np.concatenate(
        [res.results[c]["out"].astype(np.float32).reshape(NB, OUT_LEN)
         for c in range(NCORES)], axis=0
    )
    return out, res


def kernel(**inputs) -> np.ndarray:
    out, _ = _run(inputs, trace=False)
    return out
